# revision 4
# baseline (speedup 1.0000x reference)
"""ArcFace combined-margin loss kernel for 8 TRN2 NeuronCores.

Strategy
--------
reference: cos = (f @ w.T) / (|f||w|); phi = arcface(cos);
outputs = s*(labels*phi + (1-labels)*cos); loss = mean over rows of
-(sum of log_softmax(outputs) at lab_pinds, masked) / L^2.

labels is the multi-hot of (lab_pinds, lengths), so outputs differs from
s*cos only at <=8 entries/row.  The only device-scale compute is the
dense denominator  sexp[b] = sum_c exp(30*cos[b,c] - 30)  (B*C*D MACs +
B*C exps).  Everything else is O(B*L*D + C*D) and runs on host float64.

Device (per core, C-sharded: 2500 classes/core zero-padded to 2560):
  inputs are HOST-prepared fp8 operands, pre-normalized, pre-scaled and
  pre-transposed so the exp argument has a CONSTANT scale/bias:
     fT8[d, b] = fp8(30 * f[b,d] / |f_b|)      [512, 2048]
     wT8[d, c] = fp8(16 * w[c,d] / |w_c|)      [512, 2560]  (class shard)
  dot_psum = sum_d fT8*wT8 = 480*cos, so exp arg = dot/16 - 30 for every
  element -- ACT instructions need no per-row scale and can span any
  PSUM bank group.  Loop: 80 (row-block m, class-chunk n) tiles in
  block-major order; each tile = 2 fp8 DoubleRow matmuls (K=256) into
  one PSUM bank; groups of 4 tiles (4 banks, double-buffered 4+4) are
  evicted by one ACT Exp -> bf16 SBUF strip; DVE tensor_reduce sums each
  block's 2560-wide strip into sexp[128, 16].
Host (numpy float64): positive dots f.w[pinds] exactly, arcface margin,
denominator correction at positives, logsumexp, masked ragged CE, mean.
No collectives (the only cross-core reduction is summing 8 sexp
partials on host during unsharding).
"""

import math
import sys

import numpy as np
import ml_dtypes

for _p in ("/opt/trn_rl_repo",):
    if _p not in sys.path:
        sys.path.append(_p)

import concourse.bass as bass
import concourse.bacc as bacc
import concourse.mybir as mybir
import concourse.tile as tile
from concourse.bass_utils import run_bass_kernel_spmd
from contextlib import ExitStack

B, C, D, LMAX = 2048, 20000, 512, 8
NCORES = 8
CSH = C // NCORES          # 2500 real classes per core
CSHP = 2560                # padded to 5*512 (bank-aligned chunks)
NBLK = B // 128            # 16 row blocks
NW = 512                   # matmul N-chunk width (exactly one PSUM bank)
NCH = CSHP // NW           # 5 chunks per core
KC = D // 128              # 4 contraction chunks (128 partitions each)
NT = NBLK * NCH            # 80 (m, n) tiles
NG = NT // 4               # 20 ACT eviction groups of 4 banks
S = 30.0
M_MARGIN = 0.5
FSC = 30.0                 # f rows scaled to 30*unit
WSC = 16.0                 # w rows scaled to 16*unit
# psum dot = FSC*WSC*cos; exp arg = dot/WSC - 30 = 30*cos - 30

F32 = mybir.dt.float32
BF16 = mybir.dt.bfloat16
FP8 = mybir.dt.float8e4
E4M3 = ml_dtypes.float8_e4m3

_GRAPH = None


def build_graph():
    nc = bacc.Bacc()
    fT_ext = nc.declare_dram_parameter("fT8", [D, B], FP8, isOutput=False)
    wT_ext = nc.declare_dram_parameter("wT8", [D, CSHP], FP8, isOutput=False)
    sexp_ext = nc.declare_dram_parameter("sexp", [128, NBLK], F32, isOutput=True)

    AF = mybir.ActivationFunctionType

    with ExitStack() as ctx:
        tc = ctx.enter_context(tile.TileContext(nc))
        const = ctx.enter_context(tc.tile_pool(name="const", bufs=1))
        resident = ctx.enter_context(tc.tile_pool(name="resident", bufs=1))
        pmm = ctx.enter_context(tc.tile_pool(name="pmm", bufs=2, space="PSUM"))

        nbias = const.tile([128, 1], F32)
        nc.vector.memset(nbias[:], -S)

        fT = resident.tile([128, KC, B], FP8)
        wT = resident.tile([128, KC, CSHP], FP8)
        strip = resident.tile([128, NT, NW], BF16)
        sexp_t = resident.tile([128, NBLK], F32)

        # 8 full-width DMAs spread across 4 engine queues (each dma_start
        # costs ~600ns serially on its issuing queue).  First matmul pair
        # (k2=0) needs fT/wT k in {0,1} -- first on each queue; k {2,3}
        # arrive while the k2=0 matmuls run.
        def _dma(eng, t, ext, k):
            eng.dma_start(t[:, k, :], ext[k * 128 : (k + 1) * 128, :])

        _dma(nc.sync, fT, fT_ext, 0)
        _dma(nc.gpsimd, wT, wT_ext, 0)
        _dma(nc.scalar, fT, fT_ext, 1)
        _dma(nc.sync, wT, wT_ext, 1)
        _dma(nc.gpsimd, fT, fT_ext, 2)
        _dma(nc.scalar, wT, wT_ext, 3)
        _dma(nc.sync, wT, wT_ext, 2)
        _dma(nc.gpsimd, fT, fT_ext, 3)

        # main loop: tiles t = 5*m + n in block-major order; 4 banks/group
        for g in range(NG):
            P = pmm.tile([128, 4, NW], F32, tag="mm", name=f"mm{g}")
            for j in range(4):
                t = 4 * g + j
                m, n = t // NCH, t % NCH
                for k2 in range(KC // 2):
                    nc.tensor.matmul(
                        P[:, j, :],
                        fT[:, 2 * k2 : 2 * k2 + 2, m * 128 : (m + 1) * 128],
                        wT[:, 2 * k2 : 2 * k2 + 2, n * NW : (n + 1) * NW],
                        start=(k2 == 0),
                        stop=(k2 == KC // 2 - 1),
                        perf_mode=mybir.MatmulPerfMode.DoubleRow,
                    )
            nc.scalar.activation(
                strip[:, 4 * g : 4 * g + 4, :], P[:], AF.Exp,
                bias=nbias[:], scale=1.0 / WSC,
            )
            # blocks fully evicted by this group get their row-sum on DVE
            for m in range(NBLK):
                if (NCH * m + NCH - 1) // 4 == g:
                    nc.vector.tensor_reduce(
                        sexp_t[:, m : m + 1],
                        strip[:, NCH * m : NCH * m + NCH, :],
                        axis=mybir.AxisListType.XY,
                        op=mybir.AluOpType.add,
                    )
        nc.sync.dma_start(sexp_ext[:, :], sexp_t[:])

    nc.finalize()
    return nc


def _get_graph():
    global _GRAPH
    if _GRAPH is None:
        _GRAPH = build_graph()
    return _GRAPH


def make_in_maps(f, lab_word2vec, lab_pinds=None):
    f = np.asarray(f, dtype=np.float32)
    w = np.asarray(lab_word2vec, dtype=np.float32)
    fn = np.sqrt((f.astype(np.float64) ** 2).sum(axis=1))
    wn = np.sqrt((w.astype(np.float64) ** 2).sum(axis=1))
    fT8 = np.ascontiguousarray(
        (f * (FSC / fn)[:, None].astype(np.float32)).T
    ).astype(E4M3)
    w8 = (w * (WSC / wn)[:, None].astype(np.float32)).astype(E4M3)
    in_maps = []
    for i in range(NCORES):
        wT8 = np.zeros((D, CSHP), dtype=E4M3)
        wT8[:, :CSH] = w8[i * CSH : (i + 1) * CSH].T
        in_maps.append({"fT8": fT8, "wT8": wT8})
    return in_maps


def combine(outs, f, lab_word2vec, lab_pinds, lengths):
    """outs: list of 8 dicts with sexp [128, NBLK]. Returns float32 loss."""
    f = np.asarray(f, dtype=np.float64)
    w = np.asarray(lab_word2vec, dtype=np.float64)
    pinds = np.asarray(lab_pinds, dtype=np.int64)
    lens = np.asarray(lengths, dtype=np.int64)

    # s_shift[b] = sum_c exp(30 cos - 30); b = m*128 + p
    s_shift = np.zeros(B, dtype=np.float64)
    for i in range(NCORES):
        s_shift += outs[i]["sexp"].astype(np.float64).T.reshape(B)
    # the 60 zero-pad classes per core contribute exp(-30) each (cos = 0)
    s_shift -= NCORES * (CSHP - CSH) * math.exp(-S)

    fn = np.sqrt((f * f).sum(axis=1))     # [B]
    wn = np.sqrt((w * w).sum(axis=1))     # [C]
    pd = np.einsum("bjd,bd->bj", w[pinds], f)              # [B, LMAX]
    cos = pd / np.maximum(fn[:, None] * wn[pinds], 1e-8)

    cos_m, sin_m = math.cos(M_MARGIN), math.sin(M_MARGIN)
    th = math.cos(math.pi - M_MARGIN)
    mm = math.sin(math.pi - M_MARGIN) * M_MARGIN
    sine = np.sqrt(np.clip(1.0 - cos * cos, 0.0, 1.0))
    phi = cos * cos_m - sine * sin_m
    phi = np.where(cos > th, phi, cos - mm)

    mask = (np.arange(LMAX)[None, :] < lens[:, None]).astype(np.float64)
    corr = (mask * (np.exp(S * phi - S) - np.exp(S * cos - S))).sum(axis=1)
    z = S + np.log(s_shift + corr)  # logsumexp of outputs, [B]
    pos_sum = (mask * (S * phi)).sum(axis=1)
    L = lens.astype(np.float64)
    per_sample = (L * z - pos_sum) / (L * L)
    return np.float32(per_sample.mean())


def kernel(f, labels, lab_word2vec, lab_pinds, lengths):
    nc = _get_graph()
    in_maps = make_in_maps(f, lab_word2vec)
    res = run_bass_kernel_spmd(nc, in_maps, core_ids=list(range(NCORES)))
    return combine(res.results, f, lab_word2vec, lab_pinds, lengths)


# revision 7
# speedup vs baseline: 1.1609x; 1.1609x over previous
"""ArcFace combined-margin loss kernel for 8 TRN2 NeuronCores.

Strategy
--------
reference: cos = (f @ w.T) / (|f||w|); phi = arcface(cos);
outputs = s*(labels*phi + (1-labels)*cos); loss = mean over rows of
-(sum of log_softmax(outputs) at lab_pinds, masked) / L^2.

labels is the multi-hot of (lab_pinds, lengths), so outputs differs from
s*cos only at <=8 entries/row.  The only device-scale compute is the
dense denominator  sexp[b] = sum_c exp(30*cos[b,c] - 30)  (B*C*D MACs +
B*C exps).  Everything else is O(B*L*D + C*D) and runs on host float64.

Device (per core, C-sharded: 2500 classes/core zero-padded to 2560):
  inputs are HOST-prepared fp8 operands, pre-normalized, pre-scaled and
  pre-transposed so the exp argument has a CONSTANT scale/bias:
     fT8[d, b] = fp8(30 * f[b,d] / |f_b|)      [512, 2048]
     wT8[d, c] = fp8(16 * w[c,d] / |w_c|)      [512, 2560]  (class shard)
  dot_psum = sum_d fT8*wT8 = 480*cos, so exp arg = dot/16 - 30 for every
  element -- ACT instructions need no per-row scale and can span any
  PSUM bank group.  Loop: 80 (row-block m, class-chunk n) tiles in
  block-major order; each tile = 2 fp8 DoubleRow matmuls (K=256) into
  one PSUM bank; groups of 4 tiles (4 banks, double-buffered 4+4) are
  evicted by one ACT Exp -> bf16 SBUF strip; DVE tensor_reduce sums each
  block's 2560-wide strip into sexp[128, 16].
Host (numpy float64): positive dots f.w[pinds] exactly, arcface margin,
denominator correction at positives, logsumexp, masked ragged CE, mean.
No collectives (the only cross-core reduction is summing 8 sexp
partials on host during unsharding).
"""

import math
import sys

import numpy as np
import ml_dtypes

for _p in ("/opt/trn_rl_repo",):
    if _p not in sys.path:
        sys.path.append(_p)

import concourse.bass as bass
import concourse.bacc as bacc
import concourse.mybir as mybir
import concourse.tile as tile
from concourse.bass_utils import run_bass_kernel_spmd
from contextlib import ExitStack

B, C, D, LMAX = 2048, 20000, 512, 8
NCORES = 8
CSH = C // NCORES          # 2500 real classes per core
CSHP = 2560                # padded to 5*512 (bank-aligned chunks)
NBLK = B // 128            # 16 row blocks
NW = 512                   # matmul N-chunk width (exactly one PSUM bank)
NCH = CSHP // NW           # 5 chunks per core
KC = D // 128              # 4 contraction chunks (128 partitions each)
NT = NBLK * NCH            # 80 (m, n) tiles
NG = NT // 4               # 20 ACT eviction groups of 4 banks
S = 30.0
M_MARGIN = 0.5
FSC = 30.0                 # f rows scaled to 30*unit
WSC = 16.0                 # w rows scaled to 16*unit
# psum dot = FSC*WSC*cos; exp arg = dot/WSC - 30 = 30*cos - 30

F32 = mybir.dt.float32
BF16 = mybir.dt.bfloat16
FP8 = mybir.dt.float8e4
E4M3 = ml_dtypes.float8_e4m3

_GRAPH = None


def build_graph():
    nc = bacc.Bacc()
    fT_ext = nc.declare_dram_parameter("fT8", [D, B], FP8, isOutput=False)
    wT_ext = nc.declare_dram_parameter("wT8", [D, CSHP], FP8, isOutput=False)
    sexp_ext = nc.declare_dram_parameter("sexp", [128, NBLK], F32, isOutput=True)

    AF = mybir.ActivationFunctionType

    with ExitStack() as ctx:
        tc = ctx.enter_context(tile.TileContext(nc))
        const = ctx.enter_context(tc.tile_pool(name="const", bufs=1))
        resident = ctx.enter_context(tc.tile_pool(name="resident", bufs=1))
        pmm = ctx.enter_context(tc.tile_pool(name="pmm", bufs=2, space="PSUM"))
        scr = ctx.enter_context(tc.tile_pool(name="scr", bufs=2))

        nbias = const.tile([128, 1], F32)
        nc.vector.memset(nbias[:], -S)

        fT = resident.tile([128, KC, B], FP8)
        wT = resident.tile([128, KC, CSHP], FP8)
        strip = resident.tile([128, NT, NW], BF16)
        sexp_t = resident.tile([128, NBLK], F32)

        # 8 full-width DMAs spread across 4 engine queues (each dma_start
        # costs ~600ns serially on its issuing queue).  First matmul pair
        # (k2=0) needs fT/wT k in {0,1} -- first on each queue; k {2,3}
        # arrive while the k2=0 matmuls run.
        def _dma(eng, t, ext, k):
            eng.dma_start(t[:, k, :], ext[k * 128 : (k + 1) * 128, :])

        _dma(nc.sync, fT, fT_ext, 0)
        _dma(nc.gpsimd, wT, wT_ext, 0)
        _dma(nc.scalar, fT, fT_ext, 1)
        _dma(nc.sync, wT, wT_ext, 1)
        _dma(nc.gpsimd, fT, fT_ext, 2)
        _dma(nc.scalar, wT, wT_ext, 3)
        _dma(nc.sync, wT, wT_ext, 2)
        _dma(nc.gpsimd, fT, fT_ext, 3)

        # main loop: tiles t = 5*m + n in block-major order, processed in
        # super-periods of 8 tiles (= 2 ACT groups of 4 PSUM banks).
        # k2-outer within the super-period keeps the PE stationary operand
        # constant across consecutive same-block matmuls (fewer LDWEIGHTS
        # stalls); k2=0 matmuls also only need the first-wave DMAs.
        for s in range(NG // 2):
            PA = pmm.tile([128, 4, NW], F32, tag="mm", name=f"mmA{s}")
            PB = pmm.tile([128, 4, NW], F32, tag="mm", name=f"mmB{s}")
            for k2 in range(KC // 2):
                for j in range(8):
                    t = 8 * s + j
                    m, n = t // NCH, t % NCH
                    P = PA if j < 4 else PB
                    nc.tensor.matmul(
                        P[:, j % 4, :],
                        fT[:, 2 * k2 : 2 * k2 + 2, m * 128 : (m + 1) * 128],
                        wT[:, 2 * k2 : 2 * k2 + 2, n * NW : (n + 1) * NW],
                        start=(k2 == 0),
                        stop=(k2 == KC // 2 - 1),
                        perf_mode=mybir.MatmulPerfMode.DoubleRow,
                    )
            for h, P in enumerate((PA, PB)):
                g = 2 * s + h
                nc.scalar.activation(
                    strip[:, 4 * g : 4 * g + 4, :], P[:], AF.Exp,
                    bias=nbias[:], scale=1.0 / WSC,
                )
            # blocks fully evicted by this super-period get their row-sum:
            # DVE identity tensor_scalar (2x bf16 mode) with accumulator.
            for m in range(NBLK):
                if (NCH * m + NCH - 1) // 8 == s:
                    sc = scr.tile([128, NCH, NW], BF16, tag="scr")
                    nc.vector.tensor_scalar(
                        sc[:],
                        strip[:, NCH * m : NCH * m + NCH, :],
                        1.0,
                        0.0,
                        op0=mybir.AluOpType.mult,
                        op1=mybir.AluOpType.add,
                        accum_out=sexp_t[:, m : m + 1],
                    )
        nc.sync.dma_start(sexp_ext[:, :], sexp_t[:])

    nc.finalize()
    return nc


def _get_graph():
    global _GRAPH
    if _GRAPH is None:
        _GRAPH = build_graph()
    return _GRAPH


def make_in_maps(f, lab_word2vec, lab_pinds=None):
    f = np.asarray(f, dtype=np.float32)
    w = np.asarray(lab_word2vec, dtype=np.float32)
    fn = np.sqrt((f.astype(np.float64) ** 2).sum(axis=1))
    wn = np.sqrt((w.astype(np.float64) ** 2).sum(axis=1))
    fT8 = np.ascontiguousarray(
        (f * (FSC / fn)[:, None].astype(np.float32)).T
    ).astype(E4M3)
    w8 = (w * (WSC / wn)[:, None].astype(np.float32)).astype(E4M3)
    in_maps = []
    for i in range(NCORES):
        wT8 = np.zeros((D, CSHP), dtype=E4M3)
        wT8[:, :CSH] = w8[i * CSH : (i + 1) * CSH].T
        in_maps.append({"fT8": fT8, "wT8": wT8})
    return in_maps


def combine(outs, f, lab_word2vec, lab_pinds, lengths):
    """outs: list of 8 dicts with sexp [128, NBLK]. Returns float32 loss."""
    f = np.asarray(f, dtype=np.float64)
    w = np.asarray(lab_word2vec, dtype=np.float64)
    pinds = np.asarray(lab_pinds, dtype=np.int64)
    lens = np.asarray(lengths, dtype=np.int64)

    # s_shift[b] = sum_c exp(30 cos - 30); b = m*128 + p
    s_shift = np.zeros(B, dtype=np.float64)
    for i in range(NCORES):
        s_shift += outs[i]["sexp"].astype(np.float64).T.reshape(B)
    # the 60 zero-pad classes per core contribute exp(-30) each (cos = 0)
    s_shift -= NCORES * (CSHP - CSH) * math.exp(-S)

    fn = np.sqrt((f * f).sum(axis=1))     # [B]
    wn = np.sqrt((w * w).sum(axis=1))     # [C]
    pd = np.einsum("bjd,bd->bj", w[pinds], f)              # [B, LMAX]
    cos = pd / np.maximum(fn[:, None] * wn[pinds], 1e-8)

    cos_m, sin_m = math.cos(M_MARGIN), math.sin(M_MARGIN)
    th = math.cos(math.pi - M_MARGIN)
    mm = math.sin(math.pi - M_MARGIN) * M_MARGIN
    sine = np.sqrt(np.clip(1.0 - cos * cos, 0.0, 1.0))
    phi = cos * cos_m - sine * sin_m
    phi = np.where(cos > th, phi, cos - mm)

    mask = (np.arange(LMAX)[None, :] < lens[:, None]).astype(np.float64)
    corr = (mask * (np.exp(S * phi - S) - np.exp(S * cos - S))).sum(axis=1)
    z = S + np.log(s_shift + corr)  # logsumexp of outputs, [B]
    pos_sum = (mask * (S * phi)).sum(axis=1)
    L = lens.astype(np.float64)
    per_sample = (L * z - pos_sum) / (L * L)
    return np.float32(per_sample.mean())


def kernel(f, labels, lab_word2vec, lab_pinds, lengths):
    nc = _get_graph()
    in_maps = make_in_maps(f, lab_word2vec)
    res = run_bass_kernel_spmd(nc, in_maps, core_ids=list(range(NCORES)))
    return combine(res.results, f, lab_word2vec, lab_pinds, lengths)
